# revision 2
# baseline (speedup 1.0000x reference)
"""Trainium2 Bass kernel for nn_DeTokenizer (ragged EMA de-tokenizer).

Sharding: pure data parallel over batch B=8 -> one batch element per NeuronCore.

Per-core pipeline (all fp32):
  host control plane (numpy, O(B*L) index math only):
      token_mask/prob -> decay (M,) via masked compaction (exactly mirrors the
      reference's scatter-add), gather indices gidx (L,) in [0, M] where M
      points at a zero column (positions before the first masked token).
  device:
   1. transpose hidden (M,D) -> (D,M) in 128x128 PE tiles; rows pre-scaled by
      (1-decay[m]) on DVE so the transposed tiles are b[t] = (1-a[t])*x[t].
   2. EMA scan h[t] = a[t]*h[t-1] + b[t] with tensor_tensor_scan along the
      free (M) axis, per 128-partition D-chunk (8 chunks), initial = state.
      Runs in-place over the b buffer (the gather table, cols 0..M-1).
   3. gpsimd ap_gather along M: long_T[d, l] = table[d, gidx[l]].
   4. PE-transpose back to (L,D) tiles, DVE-add residual, DMA out.
  new_state = table[:, counts-1] via a tiny ap_gather (host handles counts==0).
"""

import os
import sys

import numpy as np

if "/opt/trn_rl_repo" not in sys.path:
    sys.path.insert(0, "/opt/trn_rl_repo")

import concourse.bacc as bacc
import concourse.mybir as mybir
from concourse.bass_utils import run_bass_kernel_spmd
from concourse.tile import TileContext

B, L, D, M = 8, 4096, 1024, 2048
P = 128
DC = D // P      # 8 D-chunks of 128 partitions
MT = M // P      # 16 M-tiles
NLB = 8          # gather blocks along L
GB = L // NLB    # 512 positions per gather block
TBL = M + 1      # gather-table columns; col M is all-zero
F32 = mybir.dt.float32
I16 = mybir.dt.int16

_CACHE: dict = {}
LAST_RESULTS = None  # BassKernelResults of the most recent run (for profiling)


def _build_nc():
    nc = bacc.Bacc("TRN2", num_devices=B)

    hid = nc.dram_tensor("hid", [M, D], F32, kind="ExternalInput")
    res = nc.dram_tensor("res", [L, D], F32, kind="ExternalInput")
    dcb = nc.dram_tensor("dcb", [P, M], F32, kind="ExternalInput")
    omd = nc.dram_tensor("omd", [P, MT], F32, kind="ExternalInput")
    stt = nc.dram_tensor("stt", [P, DC], F32, kind="ExternalInput")
    gix = nc.dram_tensor("gix", [P, L // 16], I16, kind="ExternalInput")
    nsx = nc.dram_tensor("nsx", [P, 1], I16, kind="ExternalInput")
    idn = nc.dram_tensor("idn", [P, P], F32, kind="ExternalInput")
    out = nc.dram_tensor("out", [L, D], F32, kind="ExternalOutput")
    nst = nc.dram_tensor("nst", [P, DC], F32, kind="ExternalOutput")

    with TileContext(nc) as tc:
        with (
            tc.tile_pool(name="const", bufs=1) as cpool,
            tc.tile_pool(name="table", bufs=1) as tpool,
            tc.tile_pool(name="hload", bufs=3) as hpool,
            tc.tile_pool(name="rload", bufs=6) as rpool,
            tc.tile_pool(name="gath", bufs=2) as gpool,
            tc.tile_pool(name="ostore", bufs=3) as opool,
        ):
            dc_sb = cpool.tile([P, M], F32, tag="dcb")
            nc.sync.dma_start(dc_sb[:], dcb[:, :])
            omd_sb = cpool.tile([P, MT], F32, tag="omd")
            nc.sync.dma_start(omd_sb[:], omd[:, :])
            st_sb = cpool.tile([P, DC], F32, tag="stt")
            nc.sync.dma_start(st_sb[:], stt[:, :])
            gx_sb = cpool.tile([P, L // 16], I16, tag="gix")
            nc.sync.dma_start(gx_sb[:], gix[:, :])
            nx_sb = cpool.tile([P, 1], I16, tag="nsx")
            nc.sync.dma_start(nx_sb[:], nsx[:, :])
            id_sb = cpool.tile([P, P], F32, tag="idn")
            nc.sync.dma_start(id_sb[:], idn[:, :])

            # gather tables, one per D-chunk; cols 0..M-1 double as the b
            # buffer that the scan consumes in place.
            tab = [tpool.tile([P, TBL], F32, tag=f"t{c}", name=f"tab{c}") for c in range(DC)]

            # stage 1: transpose-in with (1-decay) row scaling
            with tc.tile_pool(name="ps1", bufs=1, space="PSUM") as ps1:
                for mtw in range(MT // 4):
                    pts = [ps1.tile([P, 512], F32, tag=f"p{c}", name=f"pt{c}") for c in range(DC)]
                    for i in range(4):
                        mt = 4 * mtw + i
                        h = hpool.tile([P, D], F32, tag="h")
                        nc.sync.dma_start(h[:], hid[mt * P:(mt + 1) * P, :])
                        nc.vector.tensor_scalar_mul(h[:], h[:], omd_sb[:, mt:mt + 1])
                        for c in range(DC):
                            nc.tensor.transpose(
                                pts[c][:, i * P:(i + 1) * P],
                                h[:, c * P:(c + 1) * P],
                                id_sb[:],
                            )
                    for c in range(DC):
                        dst = tab[c][:, mtw * 512:(mtw + 1) * 512]
                        if c % 2 == 0:
                            nc.scalar.copy(dst, pts[c][:])
                        else:
                            nc.vector.tensor_copy(dst, pts[c][:])

            # stage 2: EMA scan (in place over the table)
            for c in range(DC):
                nc.vector.memset(tab[c][:, M:M + 1], 0.0)
                nc.vector.tensor_tensor_scan(
                    tab[c][:, 0:M],
                    dc_sb[:],
                    tab[c][:, 0:M],
                    st_sb[:, c:c + 1],
                    mybir.AluOpType.mult,
                    mybir.AluOpType.add,
                )

            # new_state: gather column counts-1 from each chunk
            nsg = cpool.tile([P, 16], F32, tag="nsg")
            nso = cpool.tile([P, DC], F32, tag="nso")
            for c in range(DC):
                nc.gpsimd.ap_gather(
                    nsg[:], tab[c][:], nx_sb[:],
                    channels=P, num_elems=TBL, d=1, num_idxs=16,
                )
                nc.scalar.copy(nso[:, c:c + 1], nsg[:, 0:1])
            nc.sync.dma_start(nst[:, :], nso[:])

            # stage 3: gather along M, transpose back, add residual
            with tc.tile_pool(name="ps3", bufs=3, space="PSUM") as ps3:
                for lb in range(NLB):
                    gs = [gpool.tile([P, GB], F32, tag=f"g{c}", name=f"g{c}") for c in range(DC)]
                    for c in range(DC):
                        nc.gpsimd.ap_gather(
                            gs[c][:], tab[c][:],
                            gx_sb[:, lb * (GB // 16):(lb + 1) * (GB // 16)],
                            channels=P, num_elems=TBL, d=1, num_idxs=GB,
                        )
                    for s in range(GB // P):
                        lt = lb * (GB // P) + s
                        r = rpool.tile([P, D], F32, tag="r")
                        nc.sync.dma_start(r[:], res[lt * P:(lt + 1) * P, :])
                        po = ps3.tile([P, D], F32, tag="po")
                        for c in range(DC):
                            nc.tensor.transpose(
                                po[:, c * P:(c + 1) * P],
                                gs[c][:, s * P:(s + 1) * P],
                                id_sb[:],
                            )
                        o = opool.tile([P, D], F32, tag="o")
                        nc.vector.tensor_add(o[:], po[:], r[:])
                        nc.sync.dma_start(out[lt * P:(lt + 1) * P, :], o[:])

    nc.compile()
    return nc


def get_nc():
    if "nc" not in _CACHE:
        _CACHE["nc"] = _build_nc()
    return _CACHE["nc"]


def _derive(token_mask, prob, counts):
    """Control-plane math, mirroring the reference bit-for-bit where values
    matter (chunk_probs scatter-add in fp32)."""
    mask_i = token_mask.astype(np.int32)
    cums = np.cumsum(mask_i, axis=1)                       # (B, L)
    pos = np.clip(cums - 1, 0, M - 1)
    chunk_probs = np.zeros((B, M), np.float32)
    pv = (prob.astype(np.float32) * token_mask.astype(np.float32))
    for b in range(B):
        np.add.at(chunk_probs[b], pos[b], pv[b])
    decay = np.clip((np.float32(1.0) - chunk_probs), 0.0, 1.0).astype(np.float32)
    gidx = np.where(cums >= 1, np.clip(cums - 1, 0, M - 1), M).astype(np.int16)
    return decay, gidx


def make_in_maps(hidden_states, residual, token_mask, prob, counts, state):
    hidden_states = np.ascontiguousarray(np.asarray(hidden_states, np.float32))
    residual = np.ascontiguousarray(np.asarray(residual, np.float32))
    token_mask = np.asarray(token_mask)
    prob = np.asarray(prob, np.float32)
    counts = np.asarray(counts, np.int32)
    state = np.asarray(state, np.float32)

    decay, gidx = _derive(token_mask, prob, counts)
    eye = np.ascontiguousarray(np.eye(P, dtype=np.float32))
    in_maps = []
    for b in range(B):
        in_maps.append({
            "hid": hidden_states[b],
            "res": residual[b],
            "dcb": np.ascontiguousarray(np.broadcast_to(decay[b], (P, M))),
            "omd": np.ascontiguousarray(
                (np.float32(1.0) - decay[b]).reshape(MT, P).T),
            "stt": np.ascontiguousarray(state[b].reshape(DC, P).T),
            "gix": np.ascontiguousarray(
                np.tile(gidx[b].reshape(L // 16, 16).T, (P // 16, 1))),
            "nsx": np.full((P, 1), max(int(counts[b]) - 1, 0), np.int16),
            "idn": eye,
        })
    return in_maps, counts, state


def assemble_outputs(results, counts, state):
    out = np.stack([results[b]["out"] for b in range(B)]).astype(np.float32)
    new_state = np.empty((B, D), np.float32)
    for b in range(B):
        if int(counts[b]) > 0:
            new_state[b] = results[b]["nst"].T.reshape(D)
        else:
            new_state[b] = state[b]
    return out, new_state


def kernel(hidden_states, residual, token_mask, prob, counts, state):
    global LAST_RESULTS
    in_maps, counts_np, state_np = make_in_maps(
        hidden_states, residual, token_mask, prob, counts, state)
    nc = get_nc()
    r = run_bass_kernel_spmd(nc, in_maps, list(range(B)))
    LAST_RESULTS = r
    return assemble_outputs(r.results, counts_np, state_np)


# revision 11
# speedup vs baseline: 4.1756x; 4.1756x over previous
"""Trainium2 Bass kernel for nn_DeTokenizer (ragged EMA de-tokenizer).

Sharding: pure data parallel over batch B=8 -> one batch element per NeuronCore.

Per-core pipeline (fp32 data path, fp32r tensor-engine matmuls):
  host control plane (numpy, O(B*L)+O(B*M*128) index/metadata math):
   - token_mask/prob -> decay a[:] (M,) exactly as the reference's scatter-add
   - per 128-chunk transfer matrices T_k[t,s] = prod_{s<r<=t} a[r] and prefix
     products E_k[t] = prod_{start<=r<=t} a[r] (built in f64 log space)
   - gather indices gidx (L,) in [0, M]; index M is an all-zero table row
  device (per chunk k of 128 M-rows, natural [M, D] layout, no transposes):
   1. b = (1-a)*x on DVE (per-partition scalar multiply)
   2. PSUM = T_k @ b + E_k (x) carry_{k-1}   (PE, fp32r: T@b as 2 N=512
      matmuls, rank-1 carry term as K=1 matmuls accumulating into the group)
   3. carry_k = PSUM row 127 (ACT copy); ema chunk -> SBUF -> DRAM table
   4. dma_gather rows table[gidx[l]] (4KB each) straight into natural-layout
      L-tiles, DVE-add residual, DMA out.
  new_state = table row counts-1 via a tiny dma_gather (host fixes counts==0).
"""

import os
import sys

import numpy as np

if "/opt/trn_rl_repo" not in sys.path:
    sys.path.insert(0, "/opt/trn_rl_repo")

import concourse.bacc as bacc
import concourse.mybir as mybir
from concourse.bass_utils import run_bass_kernel_spmd
from concourse.tile import TileContext

B, L, D, M = 8, 4096, 1024, 2048
P = 128
MT = M // P      # 16 chunks
NLB = 8          # gather blocks along L
GB = L // NLB    # 512 positions per gather block
TBL = M + 1      # table rows; row M is all-zero
F32 = mybir.dt.float32
F32R = mybir.dt.float32r
I16 = mybir.dt.int16

_CACHE: dict = {}
LAST_RESULTS = None  # BassKernelResults of the most recent run (for profiling)


def _build_nc():
    nc = bacc.Bacc("TRN2", num_devices=B)

    hid = nc.dram_tensor("hid", [M, D], F32, kind="ExternalInput")
    res = nc.dram_tensor("res", [L, D], F32, kind="ExternalInput")
    ttm = nc.dram_tensor("ttm", [P, M], F32, kind="ExternalInput")
    erw = nc.dram_tensor("erw", [1, M], F32, kind="ExternalInput")
    omd = nc.dram_tensor("omd", [P, MT], F32, kind="ExternalInput")
    els = nc.dram_tensor("els", [1, MT], F32, kind="ExternalInput")
    stn = nc.dram_tensor("stn", [1, D], F32, kind="ExternalInput")
    gix = nc.dram_tensor("gix", [P, L // 16], I16, kind="ExternalInput")
    nsx = nc.dram_tensor("nsx", [P, 1], I16, kind="ExternalInput")
    out = nc.dram_tensor("out", [L, D], F32, kind="ExternalOutput")
    nst = nc.dram_tensor("nst", [1, D], F32, kind="ExternalOutput")

    with TileContext(nc) as tc:
        with (
            tc.tile_pool(name="const", bufs=1) as cpool,
            tc.tile_pool(name="dram", bufs=1, space="DRAM") as dpool,
            tc.tile_pool(name="hload", bufs=4) as hpool,
            tc.tile_pool(name="carry", bufs=2) as carryp,
            tc.tile_pool(name="nat", bufs=3) as natp,
            tc.tile_pool(name="psum", bufs=2, space="PSUM") as psp,
            tc.tile_pool(name="rowp", bufs=2, space="PSUM") as rwp,
            tc.tile_pool(name="gath", bufs=2) as gpool,
            tc.tile_pool(name="rload", bufs=8) as rpool,
            tc.tile_pool(name="ostore", bufs=4) as opool,
        ):
            ttm_sb = cpool.tile([P, M], F32, tag="ttm")
            nc.sync.dma_start(ttm_sb[:], ttm[:, :])
            erw_sb = cpool.tile([1, M], F32, tag="erw")
            nc.sync.dma_start(erw_sb[:], erw[:, :])
            omd_sb = cpool.tile([P, MT], F32, tag="omd")
            nc.sync.dma_start(omd_sb[:], omd[:, :])
            els_sb = cpool.tile([1, MT], F32, tag="els")
            nc.sync.dma_start(els_sb[:], els[:, :])
            stn_sb = cpool.tile([1, D], F32, tag="stn")
            nc.sync.dma_start(stn_sb[:], stn[:, :])
            # fp32r-rounded copies (BIR requires matmul operands to come from
            # an op that rounds to fp32r, not straight from DMA)
            ttr_sb = cpool.tile([P, M], F32R, tag="ttr")
            nc.vector.tensor_copy(ttr_sb[:], ttm_sb[:])
            err_sb = cpool.tile([1, M], F32R, tag="err")
            nc.vector.tensor_copy(err_sb[:], erw_sb[:])
            str_sb = cpool.tile([1, D], F32R, tag="str")
            nc.vector.tensor_copy(str_sb[:], stn_sb[:])
            gx_sb = cpool.tile([P, L // 16], I16, tag="gix")
            nc.sync.dma_start(gx_sb[:], gix[:, :])
            nx_sb = cpool.tile([P, 1], I16, tag="nsx")
            nc.sync.dma_start(nx_sb[:], nsx[:, :])

            emad = dpool.tile([TBL, D], F32, tag="emad", name="emad")
            zr = cpool.tile([1, D], F32, tag="zr")
            nc.vector.memset(zr[:], 0.0)
            nc.sync.dma_start(emad[M:M + 1, :], zr[:])

            # blocked EMA scan, natural layout
            prev = str_sb
            for k in range(MT):
                h = hpool.tile([P, D], F32, tag="h")
                nc.sync.dma_start(h[:], hid[k * P:(k + 1) * P, :])
                bk = hpool.tile([P, D], F32R, tag="bk")
                nc.vector.tensor_scalar_mul(bk[:], h[:], omd_sb[:, k:k + 1])
                pk = psp.tile([P, D], F32, tag="pk", name=f"pk{k}")
                rw = rwp.tile([1, D], F32, tag="rw", name=f"rw{k}")
                lhsT = ttr_sb[:, k * P:(k + 1) * P]
                lastcol = ttr_sb[:, k * P + P - 1:k * P + P]
                elh = err_sb[0:1, k * P:(k + 1) * P]
                for hf in range(2):
                    cols = slice(hf * 512, (hf + 1) * 512)
                    nc.tensor.matmul(pk[:, cols], lhsT,
                                     bk[:, cols],
                                     start=True, stop=False)
                    nc.tensor.matmul(pk[:, cols], elh,
                                     prev[0:1, cols],
                                     start=False, stop=True)
                    # local last row of T_k @ b (independent of the carry
                    # chain, so these pipeline ahead of it)
                    nc.tensor.matmul(rw[:, cols], lastcol,
                                     bk[:, cols],
                                     start=True, stop=True)
                # carry_k = E_k[127]*carry_{k-1} + (T_k @ b)[127]
                cr = carryp.tile([1, D], F32R, tag="cr", name=f"cr{k}")
                nc.vector.scalar_tensor_tensor(
                    cr[:], prev[0:1, :], els_sb[0:1, k:k + 1], rw[:],
                    mybir.AluOpType.mult, mybir.AluOpType.add)
                prev = cr
                nt = natp.tile([P, D], F32, tag="nt")
                if k % 2 == 0:
                    nc.scalar.copy(nt[:], pk[:])
                else:
                    nc.vector.tensor_copy(nt[:], pk[:])
                nc.sync.dma_start(emad[k * P:(k + 1) * P, :], nt[:])

            # new_state: gather row counts-1 (any 16 dups; take partition 0)
            nsg = cpool.tile([P, D], F32, tag="nsg")
            nc.gpsimd.dma_gather(
                nsg[:].rearrange("p (n d) -> p n d", d=D),
                emad[:, :], nx_sb[:, :],
                num_idxs=16, num_idxs_reg=16, elem_size=D)
            nc.sync.dma_start(nst[:, :], nsg[0:1, :])

            # gather + residual add + store
            for lb in range(NLB):
                g = gpool.tile([P, (GB // P) * D], F32, tag="g")
                nc.gpsimd.dma_gather(
                    g[:].rearrange("p (n d) -> p n d", d=D),
                    emad[:, :],
                    gx_sb[:, lb * (GB // 16):(lb + 1) * (GB // 16)],
                    num_idxs=GB, num_idxs_reg=GB, elem_size=D)
                for s in range(GB // P):
                    lt = lb * (GB // P) + s
                    r = rpool.tile([P, D], F32, tag="r")
                    nc.scalar.dma_start(r[:], res[lt * P:(lt + 1) * P, :])
                    o = opool.tile([P, D], F32, tag="o")
                    nc.vector.tensor_add(o[:], g[:, s * D:(s + 1) * D], r[:])
                    eng = nc.sync if lt % 2 == 0 else nc.scalar
                    eng.dma_start(out[lt * P:(lt + 1) * P, :], o[:])

    nc.compile()
    return nc


def get_nc():
    if "nc" not in _CACHE:
        _CACHE["nc"] = _build_nc()
    return _CACHE["nc"]


def _derive(token_mask, prob):
    """Control-plane math, mirroring the reference bit-for-bit where values
    matter (chunk_probs scatter-add in fp32)."""
    mask_i = token_mask.astype(np.int32)
    cums = np.cumsum(mask_i, axis=1)                       # (B, L)
    pos = np.clip(cums - 1, 0, M - 1)
    chunk_probs = np.zeros((B, M), np.float32)
    pv = (prob.astype(np.float32) * token_mask.astype(np.float32))
    for b in range(B):
        np.add.at(chunk_probs[b], pos[b], pv[b])
    decay = np.clip((np.float32(1.0) - chunk_probs), 0.0, 1.0).astype(np.float32)
    gidx = np.where(cums >= 1, np.clip(cums - 1, 0, M - 1), M).astype(np.int16)
    return decay, gidx


def _build_T_E(decay_b):
    """T_T_all (P, M): cols [128k:128k+128) hold T_k^T with
    T_k[t, s] = prod_{s<r<=t} a[r] (lower-triangular, unit diagonal);
    E_rows (M,): E_k[t] = prod_{start<=r<=t} a[r]. Built in f64 log space."""
    a64 = decay_b.astype(np.float64)
    T_T_all = np.zeros((P, M), np.float32)
    E_rows = np.zeros(M, np.float32)
    for k in range(MT):
        la = np.log(a64[k * P:(k + 1) * P])
        cs = np.cumsum(la)
        Tk = np.exp(cs[:, None] - cs[None, :])
        Tk[np.triu_indices(P, 1)] = 0.0
        np.fill_diagonal(Tk, 1.0)
        T_T_all[:, k * P:(k + 1) * P] = Tk.T.astype(np.float32)
        E_rows[k * P:(k + 1) * P] = np.exp(cs).astype(np.float32)
    return T_T_all, E_rows


def make_in_maps(hidden_states, residual, token_mask, prob, counts, state):
    hidden_states = np.ascontiguousarray(np.asarray(hidden_states, np.float32))
    residual = np.ascontiguousarray(np.asarray(residual, np.float32))
    token_mask = np.asarray(token_mask)
    prob = np.asarray(prob, np.float32)
    counts = np.asarray(counts, np.int32)
    state = np.asarray(state, np.float32)

    decay, gidx = _derive(token_mask, prob)
    in_maps = []
    for b in range(B):
        ttm, erow = _build_T_E(decay[b])
        in_maps.append({
            "hid": hidden_states[b],
            "res": residual[b],
            "ttm": ttm,
            "erw": erow.reshape(1, M),
            "els": np.ascontiguousarray(
                erow.reshape(MT, P)[:, P - 1].reshape(1, MT)),
            "omd": np.ascontiguousarray(
                (np.float32(1.0) - decay[b]).reshape(MT, P).T),
            "stn": state[b].reshape(1, D),
            "gix": np.ascontiguousarray(
                np.tile(gidx[b].reshape(L // 16, 16).T, (P // 16, 1))),
            "nsx": np.full((P, 1), max(int(counts[b]) - 1, 0), np.int16),
        })
    return in_maps, counts, state


def assemble_outputs(results, counts, state):
    out = np.stack([results[b]["out"] for b in range(B)]).astype(np.float32)
    new_state = np.empty((B, D), np.float32)
    for b in range(B):
        if int(counts[b]) > 0:
            new_state[b] = results[b]["nst"].reshape(D)
        else:
            new_state[b] = state[b]
    return out, new_state


def kernel(hidden_states, residual, token_mask, prob, counts, state):
    global LAST_RESULTS
    in_maps, counts_np, state_np = make_in_maps(
        hidden_states, residual, token_mask, prob, counts, state)
    nc = get_nc()
    r = run_bass_kernel_spmd(nc, in_maps, list(range(B)))
    LAST_RESULTS = r
    return assemble_outputs(r.results, counts_np, state_np)
